# revision 65
# baseline (speedup 1.0000x reference)
"""Trainium2 Bass kernel: dilated causal attention + residual layernorm.

nn_CausalAttention: B=4, S=4096, F=128, H=4, D=32, dilation 4, window 8
(9 valid keys per query at offsets 0,4,...,32), masked softmax, O-proj,
residual, layernorm(eps=1e-3), gamma=1/beta=0, all biases zero.

Sharding: 8 cores = 4 batches x 2 sequence halves (2048 rows each).
In-core, positions split by residue r = s % 4 into 4 independent causal
sliding-window-9 attentions of length 512 (+8-key halo).  The host
pre-permutes x to residue-major order, pre-transposes it to [F, seq]
layout, converts everything to bf16, and un-permutes the output.

All matmuls run in bf16 (1 PE cycle/row vs 4 for fp32).  Per (residue,
block of <=120 queries): a mask matmul (identity trick) writes the -1e9
band into PSUM, then 4 per-head matmuls accumulate q.k — contraction is
the full 128 partitions with per-head zero-padded q tiles (built by
selector-scaled evacuation, no memsets), because a PSUM accumulation
group must keep a single PE tile position (the device crashes
otherwise).  Exp on ACT evacuates pairs of blocks at once to bf16; the
denominator is an ones-matmul per head strip; V is projected on the fly
per key window; oT is normalized during evacuation (reciprocal + mult
on DVE).  Phase C folds the residual into the O-projection as an
identity matmul, evacuates once per residue on ACT, computes LN stats
with DVE bn_stats/bn_aggr, and rstd = rsqrt(var+eps) runs entirely on
DVE via a quake-style bit-trick seed plus one Newton step, so a single
activation-table set (Exp/Copy) serves the whole kernel and no
1.3us table reloads occur.  Phases are software-pipelined per residue
over one unified 8-bank PSUM layout; phase C is emitted between the
next residue's block pairs to keep the PE fed.
"""

import math

import numpy as np

NUM_HEADS = 4
KEY_DIM = 32
F = 128
B = 4
S = 4096
HALF = S // 2
NR = 4                 # dilation / residue count
SR = HALF // NR        # 512 queries per (core, residue)
SRH = SR + 8           # + key halo (8 residue-space positions)
HN = 8
NEG = -1e9
EPS = 1e-3
QB = 120               # full query block
TAIL = SR - 4 * QB     # 32
N_CORES = 8

# w_all column layout
WQ0, WK0, WV0, WO0, ID0, ONES0, EPS0, SEL0 = (
    0, 128, 256, 384, 512, 640, 672, 673)
WALL_COLS = 677
# m_all column layout
MA0, MB0, MT0, IREP0 = 0, 128, 256, 384
MALL_COLS = 384 + NUM_HEADS * QB


def _build_masks():
    u = np.arange(QB)[:, None]   # query (block-local)
    m = np.arange(128)[None, :]  # key (block-local)
    band = (m >= u) & (m <= u + 8)
    mask_main = np.where(band, 0.0, NEG).astype(np.float32)          # [QB,128]
    mask_first = np.where(band & (m >= 8), 0.0, NEG).astype(np.float32)
    mask_tail = np.where(band & (u < TAIL) & (m < TAIL + 8), 0.0,
                         NEG).astype(np.float32)
    return mask_main, mask_first, mask_tail


def _bf16(a):
    import ml_dtypes
    return np.asarray(a, dtype=ml_dtypes.bfloat16)


def _host_prep(x, Wq, Wk, Wv, Wo):
    mT_main, mT_first, mT_tail = _build_masks()

    w_all = np.zeros((F, WALL_COLS), np.float32)
    w_all[:, WQ0:WQ0 + F] = Wq.reshape(F, F) / math.sqrt(KEY_DIM)
    w_all[:, WK0:WK0 + F] = Wk.reshape(F, F)
    w_all[:, WV0:WV0 + F] = Wv.reshape(F, F)
    w_all[:, WO0:WO0 + F] = Wo.reshape(F, F)
    w_all[:, ID0:ID0 + F] = np.eye(F, dtype=np.float32)
    w_all[:, ONES0:ONES0 + 32] = 1.0
    w_all[:, EPS0] = EPS
    for h in range(NUM_HEADS):
        w_all[32 * h:32 * h + 32, SEL0 + h] = 1.0   # head-strip selectors
    w_all = _bf16(w_all)

    i_rep = np.zeros((QB, NUM_HEADS, QB), np.float32)
    for h in range(NUM_HEADS):
        i_rep[:, h, :] = np.eye(QB, dtype=np.float32)

    def mall(first):
        m = np.zeros((QB, MALL_COLS), np.float32)
        m[:, MA0:MA0 + 128] = mT_first if first else mT_main
        m[:, MB0:MB0 + 128] = mT_main
        m[:, MT0:MT0 + 128] = mT_tail
        m[:, IREP0:] = i_rep.reshape(QB, NUM_HEADS * QB)
        return _bf16(m)

    m_all_first, m_all_main = mall(True), mall(False)

    maps = []
    for c in range(N_CORES):
        b, half = divmod(c, 2)
        start = half * HALF
        lo = start - 4 * HN
        full = np.zeros((4 * HN + HALF, F), np.float32)
        src = x[b, max(lo, 0):start + HALF]
        full[4 * HN + HALF - src.shape[0]:] = src
        # residue-major: xr[r, j, :] = x[b, start + 4*(j - 8) + r] (0 if OOB)
        xr = full.reshape(HN + SR, NR, F).transpose(1, 0, 2)
        # transposed: xT[f, r, j]
        xT = _bf16(np.ascontiguousarray(xr.transpose(2, 0, 1)))
        maps.append({
            "xT": xT,
            "w_all": w_all,
            "m_all": (m_all_first if half == 0 else m_all_main),
        })
    return maps


_CACHE = {}

# debug/bisect flags
USE_REARRANGE_DMA = True   # strided dram write vs per-chunk straight DMAs
USE_LN_RSTD = True         # unused (rstd now via DVE Newton)
USE_ROWPOS_QK = True       # QK strips via tile_position rows vs lo/hi tiles
USE_BN_STATS = True        # bn_stats/bn_aggr vs square-accum
PHASES = "ABC"             # which phases to build (bisect aid)
PAIR_BLOCKS = True         # share one PSUM score tile between 2 blocks


def _build_module():
    import contextlib

    import concourse.bacc as bacc
    import concourse.mybir as mybir
    import concourse.tile as tile

    fp32 = mybir.dt.float32
    bf16 = mybir.dt.bfloat16
    i32 = mybir.dt.int32
    Act = mybir.ActivationFunctionType
    Alu = mybir.AluOpType
    H = NUM_HEADS

    nc = bacc.Bacc("TRN2", target_bir_lowering=False, debug=False,
                   enable_asserts=False, num_devices=N_CORES)

    xT_d = nc.dram_tensor("xT", [F, NR, SRH], bf16, kind="ExternalInput").ap()
    w_d = nc.dram_tensor("w_all", [F, WALL_COLS], bf16,
                         kind="ExternalInput").ap()
    m_d = nc.dram_tensor("m_all", [QB, MALL_COLS], bf16,
                         kind="ExternalInput").ap()
    y_d = nc.dram_tensor("y_res", [NR, SR, F], fp32, kind="ExternalOutput").ap()

    with tile.TileContext(nc) as tc:
        with contextlib.ExitStack() as ctx:
            consts = ctx.enter_context(tc.tile_pool(name="consts", bufs=1))
            persist = ctx.enter_context(tc.tile_pool(name="persist", bufs=1))
            work = ctx.enter_context(tc.tile_pool(name="work", bufs=3))

            sb_w = consts.tile([F, WALL_COLS], bf16, tag="w")
            sb_m = consts.tile([QB, MALL_COLS], bf16, tag="m")
            nc.sync.dma_start(out=sb_w[:], in_=w_d[:])
            wq = sb_w[:, WQ0:WQ0 + F]
            wk = sb_w[:, WK0:WK0 + F]
            wv = sb_w[:, WV0:WV0 + F]
            wo = sb_w[:, WO0:WO0 + F]
            ident = sb_w[:, ID0:ID0 + F]
            ones = sb_w[:, ONES0:ONES0 + 32]
            irep = sb_m[:, IREP0:].rearrange("p (h u) -> p h u", h=H)
            # fp32 head-strip selectors (tensor_scalar mult needs fp32 APs)
            sel32 = consts.tile([F, H], fp32, tag="sel32")
            nc.vector.tensor_copy(sel32[:], sb_w[:, SEL0:SEL0 + H])

            sb_xT = [persist.tile([F, SRH], bf16, tag=f"xT{r}",
                                  name=f"xT{r}") for r in range(NR)]
            # k packed [(h,d), j]; q zero-padded per head (only strip 32h
            # nonzero) — PSUM accumulation groups must keep one PE tile
            # position, so QK contracts over all 128 partitions with the
            # padding selecting head h (device crashes otherwise)
            sb_k = [persist.tile([F, SRH], bf16, tag=f"k{r}", name=f"k{r}")
                    for r in range(NR)]
            sb_qz = [persist.tile([F, H, SRH], bf16, tag=f"qz{r}",
                                  name=f"qz{r}") for r in range(NR)]
            sb_oT = [persist.tile([F, SR], bf16, tag=f"oT{r}", name=f"oT{r}")
                     for r in range(NR)]
            # spread input DMAs across engine queues so their sequencer and
            # DGE setup latencies overlap (HWDGE is still serial)
            dma_engs = [nc.sync, nc.scalar, nc.sync, nc.scalar]
            for r in range(NR):
                dma_engs[r].dma_start(out=sb_xT[r][:], in_=xT_d[:, r, :])
                if r == 0:
                    # masks/irep aren't needed until the first mask matmul;
                    # let xT[0] through the serial HWDGE first
                    nc.scalar.dma_start(out=sb_m[:], in_=m_d[:])

            # One unified PSUM pool layout (8 banks):
            #   "ps" [128, 2, 512] f32 x2 bufs (4 banks) — q/k projection
            #        pairs in phase A, paired score tiles in phase B
            #   "small" [128, 368] f32 x2 bufs (2 banks) — projection tails
            #        in phase A; denom/V/AV in phase B
            #   "pa" [128, 4, 128] f32 x2 bufs (2 banks) — phase C
            # Phases are software-pipelined per residue (A(0) A(1) B0 C0
            # A(2) B1 C1 A(3) ...) so no pool-boundary barriers serialize
            # them.
            psB = ctx.enter_context(tc.tile_pool(name="psB", bufs=2,
                                                 space="PSUM"))
            psC = ctx.enter_context(tc.tile_pool(name="psC", bufs=2,
                                                 space="PSUM"))

            def phase_a(r):
                xT = sb_xT[r]
                pmain = psB.tile([F, 2, 512], fp32, tag="ps",
                                 name=f"pqk{r}")
                ptail = psB.tile([128, 368], fp32, tag="small",
                                 name=f"pqkt{r}")
                nc.tensor.matmul(pmain[:, 0, :], lhsT=wq, rhs=xT[:, 0:512],
                                 start=True, stop=True)
                nc.tensor.matmul(ptail[:, 0:8], lhsT=wq, rhs=xT[:, 512:SRH],
                                 start=True, stop=True)
                nc.tensor.matmul(pmain[:, 1, :], lhsT=wk, rhs=xT[:, 0:512],
                                 start=True, stop=True)
                nc.tensor.matmul(ptail[:, 8:16], lhsT=wk, rhs=xT[:, 512:SRH],
                                 start=True, stop=True)
                # k packed: plain copies; q: per-head scaled copies (the
                # selector column zeroes the other heads' strips, giving the
                # zero-padded tiles without separate memsets)
                nc.scalar.copy(out=sb_k[r][:, 0:512], in_=pmain[:, 1, :])
                nc.scalar.copy(out=sb_k[r][:, 512:SRH], in_=ptail[:, 8:16])
                for h in range(H):
                    sel = sel32[:, h:h + 1]
                    if h % 2 == 0:
                        nc.scalar.activation(sb_qz[r][:, h, 0:512],
                                             pmain[:, 0, :],
                                             Act.Copy, scale=sel)
                    else:
                        nc.vector.tensor_scalar_mul(sb_qz[r][:, h, 0:512],
                                                    pmain[:, 0, :], sel)
                # all four heads' q tails in one broadcast multiply
                nc.vector.tensor_tensor(
                    out=sb_qz[r][:, :, 512:SRH],
                    in0=ptail[:, 0:8].unsqueeze(1).broadcast_to([F, H, 8]),
                    in1=sel32[:, :].unsqueeze(2).broadcast_to([F, H, 8]),
                    op=Alu.mult)

            pend = {}

            def b_scores(r, pair):
                """mask + QK matmuls and the Exp for one block pair."""
                kT, qz = sb_k[r], sb_qz[r]
                # [128, s, 512] f32: slot stride = one 2KB bank
                psraw = psB.tile([128, 2, 512], fp32, tag="ps")
                psv = psraw[:, :, 0:H * QB].rearrange(
                    "p s (h u) -> p s h u", h=H)
                pS = work.tile([128, 2, H, QB], bf16, tag="pS")
                for si, blk in enumerate(pair):
                    q0 = QB * blk
                    qn = QB if blk < 4 else TAIL
                    kn = 128 if blk < 4 else TAIL + 8
                    mT = sb_m[:, (MA0 if blk == 0 else
                                  (MB0 if blk < 4 else MT0)):][:, 0:kn]
                    # tail writes the full u-range (strided PSUM output
                    # is not allowed); host mask has -1e9 in columns
                    # u >= TAIL so the extras exp to 0
                    nc.tensor.matmul(psv[0:kn, si, :, :], lhsT=mT,
                                     rhs=irep[:, :, :],
                                     start=True, stop=False,
                                     skip_group_check=True)
                    for h in range(H):
                        nc.tensor.matmul(
                            psv[0:kn, si, h, 0:qn],
                            lhsT=kT[:, q0:q0 + kn],
                            rhs=qz[:, h, HN + q0:HN + q0 + qn],
                            start=False, stop=(h == H - 1),
                            skip_group_check=True)
                if pair[-1] < 4:
                    nc.scalar.activation(pS[:], psv[:], Act.Exp)
                else:
                    nc.scalar.activation(pS[0:40, 0, :, 0:TAIL],
                                         psv[0:40, 0, :, 0:TAIL],
                                         Act.Exp)
                pend[(r, pair)] = pS

            def b_post(r, pair):
                """V-proj, denominators, AV and normalize-evacuate for a
                pair whose Exp was issued one stage earlier."""
                xT, oT = sb_xT[r], sb_oT[r]
                pS = pend.pop((r, pair))
                for si, blk in enumerate(pair):
                    q0 = QB * blk
                    qn = QB if blk < 4 else TAIL
                    kn = 128 if blk < 4 else TAIL + 8
                    small = psB.tile([128, 368], fp32, tag="small")
                    pdnr = small[:, 0:120]
                    pv = small[:, 120:248]
                    po = small[:, 248:368]
                    # pv first: its evacuation overlaps the denominator
                    # matmuls so the AV matmuls don't wait on vb
                    nc.tensor.matmul(pv[0:kn, :],
                                     lhsT=xT[:, q0:q0 + kn],
                                     rhs=wv, start=True, stop=True)
                    vb = work.tile([128, F], bf16, tag="vb")
                    if si == 0:
                        nc.scalar.copy(out=vb[0:kn, :], in_=pv[0:kn, :])
                    else:
                        nc.vector.tensor_copy(vb[0:kn, :], pv[0:kn, :])
                    for h in range(H):
                        nc.tensor.matmul(
                            pdnr[32 * h:32 * h + 32, 0:qn],
                            lhsT=ones[0:kn, :],
                            rhs=pS[0:kn, si, h, 0:qn],
                            start=True, stop=True,
                            tile_position=(0, 32 * h))
                    for h in range(H):
                        nc.tensor.matmul(
                            po[32 * h:32 * h + 32, 0:qn],
                            lhsT=vb[0:kn, 32 * h:32 * h + 32],
                            rhs=pS[0:kn, si, h, 0:qn],
                            start=True, stop=True,
                            tile_position=(0, 32 * h))
                    rep = work.tile([128, QB], fp32, tag="rep")
                    nc.vector.reciprocal_approx_fast(
                        out=rep[:, 0:qn], in_=pdnr[:, 0:qn])
                    nc.vector.tensor_mul(oT[:, q0:q0 + qn],
                                         po[:, 0:qn], rep[:, 0:qn])

            def phase_c(r):
                xT, oT = sb_xT[r], sb_oT[r]
                bn6 = work.tile([128, 4, 6], fp32, tag="bn6")
                bnag = work.tile([128, 4, 2], fp32, tag="bnag")
                y0 = work.tile([128, 4, F], bf16, tag="y0", bufs=2)
                y = work.tile([128, 4, F], fp32, tag="y", bufs=2)
                pa4 = psC.tile([128, 4, F], fp32, tag="pa", name=f"pa{r}")
                for c in range(4):
                    pa = pa4[:, c, :]
                    nc.tensor.matmul(pa, lhsT=oT[:, 128 * c:128 * (c + 1)],
                                     rhs=wo, start=True, stop=False,
                                     skip_group_check=True)
                    nc.tensor.matmul(
                        pa, lhsT=xT[:, HN + 128 * c:HN + 128 * (c + 1)],
                        rhs=ident, start=False, stop=True,
                        skip_group_check=True)
                nc.scalar.copy(out=y0[:], in_=pa4[:])
                for c in range(4):
                    nc.vector.bn_stats(bn6[:, c, :], y0[:, c, :])
                    nc.vector.bn_aggr(bnag[:, c, :], bn6[:, c, :])
                # rstd = rsqrt(var + eps) on DVE only (any ACT sqrt/ln
                # would force a 1.3us activation-table reload per use):
                # quake bit-trick seed + 2 Newton steps
                vpe = work.tile([128, 4], fp32, tag="vpe")
                seed = work.tile([128, 4], fp32, tag="seed")
                nt = work.tile([128, 4], fp32, tag="nt")
                nc.vector.tensor_scalar_add(vpe[:], bnag[:, :, 1], EPS)
                # seed = MAGIC - (v >> 1) via shift, ~x, then +(MAGIC+1)
                # (one op class per instruction)
                nc.vector.tensor_single_scalar(
                    seed[:].bitcast(i32), vpe[:].bitcast(i32), 1,
                    Alu.arith_shift_right)
                nc.vector.tensor_single_scalar(
                    seed[:].bitcast(i32), seed[:].bitcast(i32), -1,
                    Alu.bitwise_xor)
                nc.vector.tensor_single_scalar(
                    seed[:].bitcast(i32), seed[:].bitcast(i32),
                    0x5F3759DF + 1, Alu.add)
                for _ in range(1):
                    nc.vector.tensor_mul(nt[:], seed[:], seed[:])
                    nc.vector.tensor_mul(nt[:], nt[:], vpe[:])
                    nc.vector.tensor_scalar(
                        out=nt[:], in0=nt[:], scalar1=-0.5, scalar2=1.5,
                        op0=Alu.mult, op1=Alu.add)
                    nc.vector.tensor_mul(seed[:], seed[:], nt[:])
                rstd = seed
                yv = y_d[r].rearrange("(c p) f -> p c f", c=4)
                for c in range(4):
                    nc.vector.tensor_scalar(
                        out=y[:, c, :], in0=y0[:, c, :],
                        scalar1=bnag[:, c, 0:1], scalar2=rstd[:, c:c + 1],
                        op0=Alu.subtract, op1=Alu.mult)
                    if c == 1:
                        # ship the first half early to shorten the end tail
                        nc.sync.dma_start(out=yv[:, 0:2, :], in_=y[:, 0:2, :])
                nc.sync.dma_start(out=yv[:, 2:4, :], in_=y[:, 2:4, :])

            # Software-pipelined flat schedule: scores(pair n+1) is emitted
            # before post(pair n) so the PE has score matmuls queued while
            # the ACT Exp for pair n completes; phase C lands one stage
            # after its residue's last post so the DVE finishes the final
            # oT normalization first; A(r+1) is emitted at each residue
            # boundary.
            stages = [(r, p) for r in range(NR)
                      for p in ((0, 1), (2, 3), (4,))]
            phase_a(0)
            phase_a(1)
            b_scores(*stages[0])
            pending_c = None
            for i in range(1, len(stages)):
                r_i, p_i = stages[i]
                if p_i == (0, 1) and r_i + 1 < NR:
                    phase_a(r_i + 1)
                b_scores(r_i, p_i)
                if pending_c is not None:
                    phase_c(pending_c)
                    pending_c = None
                r_t, p_t = stages[i - 1]
                b_post(r_t, p_t)
                if p_t == (4,):
                    pending_c = r_t
            b_post(*stages[-1])
            phase_c(NR - 1)

    nc.compile()
    return nc


def kernel(x, Wq, bq, Wk, bk, Wv, bv, Wo, bo, gamma, beta):
    from concourse.bass_utils import run_bass_kernel_spmd
    x = np.asarray(x, np.float32)
    if "nc" not in _CACHE:
        _CACHE["nc"] = _build_module()
    nc = _CACHE["nc"]
    maps = _host_prep(x, np.asarray(Wq), np.asarray(Wk),
                      np.asarray(Wv), np.asarray(Wo))
    res = run_bass_kernel_spmd(nc, maps, list(range(N_CORES)))
    out = np.zeros((B, S, F), np.float32)
    for c in range(N_CORES):
        b, half = divmod(c, 2)
        yr = res.results[c]["y_res"]                      # [NR, SR, F]
        out[b, half * HALF:(half + 1) * HALF] = (
            yr.transpose(1, 0, 2).reshape(HALF, F))
    return out


# revision 67
# speedup vs baseline: 1.1078x; 1.1078x over previous
"""Trainium2 Bass kernel: dilated causal attention + residual layernorm.

nn_CausalAttention: B=4, S=4096, F=128, H=4, D=32, dilation 4, window 8
(9 valid keys per query at offsets 0,4,...,32), masked softmax, O-proj,
residual, layernorm(eps=1e-3), gamma=1/beta=0, all biases zero.

Sharding: 8 cores = 4 batches x 2 sequence halves (2048 rows each).
In-core, positions split by residue r = s % 4 into 4 independent causal
sliding-window-9 attentions of length 512 (+8-key halo).  The host
pre-permutes x to residue-major order, pre-transposes it to [F, seq]
layout, converts everything to bf16, and un-permutes the output.

All matmuls run in bf16 (1 PE cycle/row vs 4 for fp32).  Per (residue,
block of <=120 queries): a mask matmul (identity trick) writes the -1e9
band into PSUM, then 4 per-head matmuls accumulate q.k — contraction is
the full 128 partitions with per-head zero-padded q tiles (built by
selector-scaled evacuation, no memsets), because a PSUM accumulation
group must keep a single PE tile position (the device crashes
otherwise).  Exp on ACT evacuates pairs of blocks at once to bf16; the
denominator is an ones-matmul per head strip; V is projected on the fly
per key window; oT is normalized during evacuation (reciprocal + mult
on DVE).  Phase C folds the residual into the O-projection as an
identity matmul, evacuates once per residue on ACT, computes LN stats
with DVE bn_stats/bn_aggr, and rstd = rsqrt(var+eps) runs entirely on
DVE via a quake-style bit-trick seed plus one Newton step, so a single
activation-table set (Exp/Copy) serves the whole kernel and no
1.3us table reloads occur.  Phases are software-pipelined per residue
over one unified 8-bank PSUM layout; phase C is emitted between the
next residue's block pairs to keep the PE fed.
"""

import math

import numpy as np

NUM_HEADS = 4
KEY_DIM = 32
F = 128
B = 4
S = 4096
HALF = S // 2
NR = 4                 # dilation / residue count
SR = HALF // NR        # 512 queries per (core, residue)
SRH = SR + 8           # + key halo (8 residue-space positions)
HN = 8
NEG = -1e9
EPS = 1e-3
QB = 120               # full query block
TAIL = SR - 4 * QB     # 32
N_CORES = 8

# w_all column layout
WQ0, WK0, WV0, WO0, ID0, ONES0, EPS0, SEL0 = (
    0, 128, 256, 384, 512, 640, 672, 673)
WALL_COLS = 677
# m_all column layout
MA0, MB0, MT0, IREP0 = 0, 128, 256, 384
MALL_COLS = 384 + NUM_HEADS * QB


def _build_masks():
    u = np.arange(QB)[:, None]   # query (block-local)
    m = np.arange(128)[None, :]  # key (block-local)
    band = (m >= u) & (m <= u + 8)
    mask_main = np.where(band, 0.0, NEG).astype(np.float32)          # [QB,128]
    mask_first = np.where(band & (m >= 8), 0.0, NEG).astype(np.float32)
    mask_tail = np.where(band & (u < TAIL) & (m < TAIL + 8), 0.0,
                         NEG).astype(np.float32)
    return mask_main, mask_first, mask_tail


def _bf16(a):
    import ml_dtypes
    return np.asarray(a, dtype=ml_dtypes.bfloat16)


def _host_prep(x, Wq, Wk, Wv, Wo):
    mT_main, mT_first, mT_tail = _build_masks()

    w_all = np.zeros((F, WALL_COLS), np.float32)
    w_all[:, WQ0:WQ0 + F] = Wq.reshape(F, F) / math.sqrt(KEY_DIM)
    w_all[:, WK0:WK0 + F] = Wk.reshape(F, F)
    w_all[:, WV0:WV0 + F] = Wv.reshape(F, F)
    w_all[:, WO0:WO0 + F] = Wo.reshape(F, F)
    w_all[:, ID0:ID0 + F] = np.eye(F, dtype=np.float32)
    w_all[:, ONES0:ONES0 + 32] = 1.0
    w_all[:, EPS0] = EPS
    for h in range(NUM_HEADS):
        w_all[32 * h:32 * h + 32, SEL0 + h] = 1.0   # head-strip selectors
    w_all = _bf16(w_all)

    i_rep = np.zeros((QB, NUM_HEADS, QB), np.float32)
    for h in range(NUM_HEADS):
        i_rep[:, h, :] = np.eye(QB, dtype=np.float32)

    def mall(first):
        m = np.zeros((QB, MALL_COLS), np.float32)
        m[:, MA0:MA0 + 128] = mT_first if first else mT_main
        m[:, MB0:MB0 + 128] = mT_main
        m[:, MT0:MT0 + 128] = mT_tail
        m[:, IREP0:] = i_rep.reshape(QB, NUM_HEADS * QB)
        return _bf16(m)

    m_all_first, m_all_main = mall(True), mall(False)

    maps = []
    for c in range(N_CORES):
        b, half = divmod(c, 2)
        start = half * HALF
        lo = start - 4 * HN
        full = np.zeros((4 * HN + HALF, F), np.float32)
        src = x[b, max(lo, 0):start + HALF]
        full[4 * HN + HALF - src.shape[0]:] = src
        # residue-major: xr[r, j, :] = x[b, start + 4*(j - 8) + r] (0 if OOB)
        xr = full.reshape(HN + SR, NR, F).transpose(1, 0, 2)
        # transposed: xT[f, r, j]
        xT = _bf16(np.ascontiguousarray(xr.transpose(2, 0, 1)))
        maps.append({
            "xT": xT,
            "w_all": w_all,
            "m_all": (m_all_first if half == 0 else m_all_main),
        })
    return maps


_CACHE = {}

# debug/bisect flags
USE_REARRANGE_DMA = True   # strided dram write vs per-chunk straight DMAs
USE_LN_RSTD = True         # unused (rstd now via DVE Newton)
USE_ROWPOS_QK = True       # QK strips via tile_position rows vs lo/hi tiles
USE_BN_STATS = True        # bn_stats/bn_aggr vs square-accum
PHASES = "ABC"             # which phases to build (bisect aid)
PAIR_BLOCKS = True         # share one PSUM score tile between 2 blocks


def _build_module():
    import contextlib

    import concourse.bacc as bacc
    import concourse.mybir as mybir
    import concourse.tile as tile

    fp32 = mybir.dt.float32
    bf16 = mybir.dt.bfloat16
    i32 = mybir.dt.int32
    Act = mybir.ActivationFunctionType
    Alu = mybir.AluOpType
    H = NUM_HEADS

    nc = bacc.Bacc("TRN2", target_bir_lowering=False, debug=False,
                   enable_asserts=False, num_devices=N_CORES)

    xT_d = nc.dram_tensor("xT", [F, NR, SRH], bf16, kind="ExternalInput").ap()
    w_d = nc.dram_tensor("w_all", [F, WALL_COLS], bf16,
                         kind="ExternalInput").ap()
    m_d = nc.dram_tensor("m_all", [QB, MALL_COLS], bf16,
                         kind="ExternalInput").ap()
    y_d = nc.dram_tensor("y_res", [NR, SR, F], fp32, kind="ExternalOutput").ap()

    with tile.TileContext(nc) as tc:
        with contextlib.ExitStack() as ctx:
            consts = ctx.enter_context(tc.tile_pool(name="consts", bufs=1))
            persist = ctx.enter_context(tc.tile_pool(name="persist", bufs=1))
            work = ctx.enter_context(tc.tile_pool(name="work", bufs=3))

            sb_w = consts.tile([F, WALL_COLS], bf16, tag="w")
            sb_m = consts.tile([QB, MALL_COLS], bf16, tag="m")
            nc.sync.dma_start(out=sb_w[:], in_=w_d[:])
            wq = sb_w[:, WQ0:WQ0 + F]
            wk = sb_w[:, WK0:WK0 + F]
            wv = sb_w[:, WV0:WV0 + F]
            wo = sb_w[:, WO0:WO0 + F]
            ident = sb_w[:, ID0:ID0 + F]
            ones = sb_w[:, ONES0:ONES0 + 32]
            irep = sb_m[:, IREP0:].rearrange("p (h u) -> p h u", h=H)
            # fp32 head-strip selectors (tensor_scalar mult needs fp32 APs)
            sel32 = consts.tile([F, H], fp32, tag="sel32")
            nc.vector.tensor_copy(sel32[:], sb_w[:, SEL0:SEL0 + H])

            sb_xT = [persist.tile([F, SRH], bf16, tag=f"xT{r}",
                                  name=f"xT{r}") for r in range(NR)]
            # k packed [(h,d), j]; q zero-padded per head (only strip 32h
            # nonzero) — PSUM accumulation groups must keep one PE tile
            # position, so QK contracts over all 128 partitions with the
            # padding selecting head h (device crashes otherwise)
            sb_k = [persist.tile([F, SRH], bf16, tag=f"k{r}", name=f"k{r}")
                    for r in range(NR)]
            sb_qz = [persist.tile([F, H, SRH], bf16, tag=f"qz{r}",
                                  name=f"qz{r}") for r in range(NR)]
            sb_oT = [persist.tile([F, SR], bf16, tag=f"oT{r}", name=f"oT{r}")
                     for r in range(NR)]
            # spread input DMAs across engine queues so their sequencer and
            # DGE setup latencies overlap (HWDGE is still serial)
            dma_engs = [nc.sync, nc.scalar, nc.sync, nc.scalar]
            for r in range(NR):
                dma_engs[r].dma_start(out=sb_xT[r][:], in_=xT_d[:, r, :])
                if r == 0:
                    # masks/irep aren't needed until the first mask matmul;
                    # let xT[0] through the serial HWDGE first
                    nc.scalar.dma_start(out=sb_m[:], in_=m_d[:])

            # One unified PSUM pool layout (8 banks):
            #   "ps" [128, 2, 512] f32 x2 bufs (4 banks) — q/k projection
            #        pairs in phase A, paired score tiles in phase B
            #   "small" [128, 368] f32 x2 bufs (2 banks) — projection tails
            #        in phase A; denom/V/AV in phase B
            #   "pa" [128, 4, 128] f32 x2 bufs (2 banks) — phase C
            # Phases are software-pipelined per residue (A(0) A(1) B0 C0
            # A(2) B1 C1 A(3) ...) so no pool-boundary barriers serialize
            # them.
            psB = ctx.enter_context(tc.tile_pool(name="psB", bufs=2,
                                                 space="PSUM"))
            psC = ctx.enter_context(tc.tile_pool(name="psC", bufs=1,
                                                 space="PSUM"))

            def phase_a(r):
                xT = sb_xT[r]
                pmain = psB.tile([F, 2, 512], fp32, tag="ps",
                                 name=f"pqk{r}")
                ptail = psB.tile([128, 368], fp32, tag="small",
                                 name=f"pqkt{r}", bufs=3)
                nc.tensor.matmul(pmain[:, 0, :], lhsT=wq, rhs=xT[:, 0:512],
                                 start=True, stop=True)
                nc.tensor.matmul(ptail[:, 0:8], lhsT=wq, rhs=xT[:, 512:SRH],
                                 start=True, stop=True)
                nc.tensor.matmul(pmain[:, 1, :], lhsT=wk, rhs=xT[:, 0:512],
                                 start=True, stop=True)
                nc.tensor.matmul(ptail[:, 8:16], lhsT=wk, rhs=xT[:, 512:SRH],
                                 start=True, stop=True)
                # k packed: plain copies; q: per-head scaled copies (the
                # selector column zeroes the other heads' strips, giving the
                # zero-padded tiles without separate memsets)
                nc.scalar.copy(out=sb_k[r][:, 0:512], in_=pmain[:, 1, :])
                nc.scalar.copy(out=sb_k[r][:, 512:SRH], in_=ptail[:, 8:16])
                for h in range(H):
                    sel = sel32[:, h:h + 1]
                    if h % 2 == 0:
                        nc.scalar.activation(sb_qz[r][:, h, 0:512],
                                             pmain[:, 0, :],
                                             Act.Copy, scale=sel)
                    else:
                        nc.vector.tensor_scalar_mul(sb_qz[r][:, h, 0:512],
                                                    pmain[:, 0, :], sel)
                # all four heads' q tails in one broadcast multiply
                nc.vector.tensor_tensor(
                    out=sb_qz[r][:, :, 512:SRH],
                    in0=ptail[:, 0:8].unsqueeze(1).broadcast_to([F, H, 8]),
                    in1=sel32[:, :].unsqueeze(2).broadcast_to([F, H, 8]),
                    op=Alu.mult)

            pend = {}

            def b_scores(r, pair):
                """mask + QK matmuls and the Exp for one block pair."""
                kT, qz = sb_k[r], sb_qz[r]
                # [128, s, 512] f32: slot stride = one 2KB bank
                psraw = psB.tile([128, 2, 512], fp32, tag="ps")
                psv = psraw[:, :, 0:H * QB].rearrange(
                    "p s (h u) -> p s h u", h=H)
                pS = work.tile([128, 2, H, QB], bf16, tag="pS", bufs=4)
                for si, blk in enumerate(pair):
                    q0 = QB * blk
                    qn = QB if blk < 4 else TAIL
                    kn = 128 if blk < 4 else TAIL + 8
                    mT = sb_m[:, (MA0 if blk == 0 else
                                  (MB0 if blk < 4 else MT0)):][:, 0:kn]
                    # tail writes the full u-range (strided PSUM output
                    # is not allowed); host mask has -1e9 in columns
                    # u >= TAIL so the extras exp to 0
                    nc.tensor.matmul(psv[0:kn, si, :, :], lhsT=mT,
                                     rhs=irep[:, :, :],
                                     start=True, stop=False,
                                     skip_group_check=True)
                    for h in range(H):
                        nc.tensor.matmul(
                            psv[0:kn, si, h, 0:qn],
                            lhsT=kT[:, q0:q0 + kn],
                            rhs=qz[:, h, HN + q0:HN + q0 + qn],
                            start=False, stop=(h == H - 1),
                            skip_group_check=True)
                if pair[-1] < 4:
                    nc.scalar.activation(pS[:], psv[:], Act.Exp)
                else:
                    nc.scalar.activation(pS[0:40, 0, :, 0:TAIL],
                                         psv[0:40, 0, :, 0:TAIL],
                                         Act.Exp)
                pend[(r, pair)] = pS

            def b_post(r, pair):
                """V-proj, denominators, AV and normalize-evacuate for a
                pair whose Exp was issued one stage earlier."""
                xT, oT = sb_xT[r], sb_oT[r]
                pS = pend.pop((r, pair))
                for si, blk in enumerate(pair):
                    q0 = QB * blk
                    qn = QB if blk < 4 else TAIL
                    kn = 128 if blk < 4 else TAIL + 8
                    small = psB.tile([128, 368], fp32, tag="small",
                                     bufs=3)
                    pdnr = small[:, 0:120]
                    pv = small[:, 120:248]
                    po = small[:, 248:368]
                    # pv first: its evacuation overlaps the denominator
                    # matmuls so the AV matmuls don't wait on vb
                    nc.tensor.matmul(pv[0:kn, :],
                                     lhsT=xT[:, q0:q0 + kn],
                                     rhs=wv, start=True, stop=True)
                    vb = work.tile([128, F], bf16, tag="vb")
                    if si == 0:
                        nc.scalar.copy(out=vb[0:kn, :], in_=pv[0:kn, :])
                    else:
                        nc.vector.tensor_copy(vb[0:kn, :], pv[0:kn, :])
                    for h in range(H):
                        nc.tensor.matmul(
                            pdnr[32 * h:32 * h + 32, 0:qn],
                            lhsT=ones[0:kn, :],
                            rhs=pS[0:kn, si, h, 0:qn],
                            start=True, stop=True,
                            tile_position=(0, 32 * h))
                    for h in range(H):
                        nc.tensor.matmul(
                            po[32 * h:32 * h + 32, 0:qn],
                            lhsT=vb[0:kn, 32 * h:32 * h + 32],
                            rhs=pS[0:kn, si, h, 0:qn],
                            start=True, stop=True,
                            tile_position=(0, 32 * h))
                    rep = work.tile([128, QB], fp32, tag="rep")
                    nc.vector.reciprocal_approx_fast(
                        out=rep[:, 0:qn], in_=pdnr[:, 0:qn])
                    nc.vector.tensor_mul(oT[:, q0:q0 + qn],
                                         po[:, 0:qn], rep[:, 0:qn])

            def phase_c(r):
                xT, oT = sb_xT[r], sb_oT[r]
                bn6 = work.tile([128, 4, 6], fp32, tag="bn6")
                bnag = work.tile([128, 4, 2], fp32, tag="bnag")
                y0 = work.tile([128, 4, F], bf16, tag="y0", bufs=2)
                y = work.tile([128, 4, F], fp32, tag="y", bufs=2)
                pa4 = psC.tile([128, 4, F], fp32, tag="pa", name=f"pa{r}")
                for c in range(4):
                    pa = pa4[:, c, :]
                    nc.tensor.matmul(pa, lhsT=oT[:, 128 * c:128 * (c + 1)],
                                     rhs=wo, start=True, stop=False,
                                     skip_group_check=True)
                    nc.tensor.matmul(
                        pa, lhsT=xT[:, HN + 128 * c:HN + 128 * (c + 1)],
                        rhs=ident, start=False, stop=True,
                        skip_group_check=True)
                nc.scalar.copy(out=y0[:], in_=pa4[:])
                for c in range(4):
                    nc.vector.bn_stats(bn6[:, c, :], y0[:, c, :])
                    nc.vector.bn_aggr(bnag[:, c, :], bn6[:, c, :])
                # rstd = rsqrt(var + eps) on DVE only (any ACT sqrt/ln
                # would force a 1.3us activation-table reload per use):
                # quake bit-trick seed + 2 Newton steps
                vpe = work.tile([128, 4], fp32, tag="vpe")
                seed = work.tile([128, 4], fp32, tag="seed")
                nt = work.tile([128, 4], fp32, tag="nt")
                nc.vector.tensor_scalar_add(vpe[:], bnag[:, :, 1], EPS)
                # seed = MAGIC - (v >> 1) via shift, ~x, then +(MAGIC+1)
                # (one op class per instruction)
                nc.vector.tensor_single_scalar(
                    seed[:].bitcast(i32), vpe[:].bitcast(i32), 1,
                    Alu.arith_shift_right)
                nc.vector.tensor_single_scalar(
                    seed[:].bitcast(i32), seed[:].bitcast(i32), -1,
                    Alu.bitwise_xor)
                nc.vector.tensor_single_scalar(
                    seed[:].bitcast(i32), seed[:].bitcast(i32),
                    0x5F3759DF + 1, Alu.add)
                for _ in range(1):
                    nc.vector.tensor_mul(nt[:], seed[:], seed[:])
                    nc.vector.tensor_mul(nt[:], nt[:], vpe[:])
                    nc.vector.tensor_scalar(
                        out=nt[:], in0=nt[:], scalar1=-0.5, scalar2=1.5,
                        op0=Alu.mult, op1=Alu.add)
                    nc.vector.tensor_mul(seed[:], seed[:], nt[:])
                rstd = seed
                yv = y_d[r].rearrange("(c p) f -> p c f", c=4)
                for c in range(4):
                    nc.vector.tensor_scalar(
                        out=y[:, c, :], in0=y0[:, c, :],
                        scalar1=bnag[:, c, 0:1], scalar2=rstd[:, c:c + 1],
                        op0=Alu.subtract, op1=Alu.mult)
                    if c == 1:
                        # ship the first half early to shorten the end tail
                        nc.sync.dma_start(out=yv[:, 0:2, :], in_=y[:, 0:2, :])
                nc.sync.dma_start(out=yv[:, 2:4, :], in_=y[:, 2:4, :])

            # Software-pipelined flat schedule: scores(pair n+1) is emitted
            # before post(pair n) so the PE has score matmuls queued while
            # the ACT Exp for pair n completes; phase C lands one stage
            # after its residue's last post so the DVE finishes the final
            # oT normalization first; A(r+1) is emitted at each residue
            # boundary.
            stages = [(r, p) for r in range(NR)
                      for p in ((0, 1), (2, 3), (4,))]
            phase_a(0)
            phase_a(1)
            b_scores(*stages[0])
            pending_c = None
            for i in range(1, len(stages)):
                r_i, p_i = stages[i]
                if p_i == (0, 1) and r_i + 1 < NR:
                    phase_a(r_i + 1)
                b_scores(r_i, p_i)
                if pending_c is not None:
                    phase_c(pending_c)
                    pending_c = None
                r_t, p_t = stages[i - 1]
                b_post(r_t, p_t)
                if p_t == (4,):
                    pending_c = r_t
            b_post(*stages[-1])
            phase_c(NR - 1)

    nc.compile()
    return nc


def kernel(x, Wq, bq, Wk, bk, Wv, bv, Wo, bo, gamma, beta):
    from concourse.bass_utils import run_bass_kernel_spmd
    x = np.asarray(x, np.float32)
    if "nc" not in _CACHE:
        _CACHE["nc"] = _build_module()
    nc = _CACHE["nc"]
    maps = _host_prep(x, np.asarray(Wq), np.asarray(Wk),
                      np.asarray(Wv), np.asarray(Wo))
    res = run_bass_kernel_spmd(nc, maps, list(range(N_CORES)))
    out = np.zeros((B, S, F), np.float32)
    for c in range(N_CORES):
        b, half = divmod(c, 2)
        yr = res.results[c]["y_res"]                      # [NR, SR, F]
        out[b, half * HALF:(half + 1) * HALF] = (
            yr.transpose(1, 0, 2).reshape(HALF, F))
    return out
